# revision 8
# baseline (speedup 1.0000x reference)
"""Multi-head attention (B=2, S=2048, D=1024, H=16) on 8 trn2 NeuronCores.

Sharding: head-parallel. Core c owns heads {2c, 2c+1} (= feature rows
[128c, 128c+128) of the QKV projections / columns of Wo). Single fully
software-pipelined instruction stream per core:
  - projection chunks (512 tokens) of batch 1, and the fc output
    projection of already-finished q-chunks, are spliced into the
    attention kt-loops of earlier chunks so the PE never drains;
  - attention runs per (batch, 512-wide q-chunk): scores (K padded to
    128 with zero rows), exp on ACT, AV accumulation against a
    [V_h0 | ones | V_h1] block (matmul cost depends only on the moving
    dim, so the 64 ones columns are free) that yields the softmax
    denominator pre-broadcast across 64 partitions;
  - normalization is then fully on-chip: lane-parallel reciprocal,
    one half-swap identity matmul to move the reciprocals onto the
    head's partitions, and a multiply straight into attnT.
Output partials are bf16; host sums the 8 partials and adds the
(bo + bv @ Wo.T) constant (where the bv bias lands after softmax
normalization algebra).
"""

import sys

for _p in ("/opt/trn_rl_repo",):
    if _p not in sys.path:
        try:
            import concourse  # noqa: F401
            break
        except ImportError:
            sys.path.insert(0, _p)

import numpy as np
import ml_dtypes

import concourse.bass as bass
import concourse.tile as tile
from concourse import mybir
from concourse.bass_utils import run_bass_kernel_spmd

BF16 = mybir.dt.bfloat16
F32 = mybir.dt.float32
AF = mybir.ActivationFunctionType

B, S, D, H, DH = 2, 2048, 1024, 16, 64
NCORES = 8
T = B * S              # 4096 tokens
HC = 128               # head-columns per core (2 heads x 64)
KO = D // 128          # 8 contraction tiles for projections
SCALE = DH ** -0.5     # 0.125
CH = 512               # projection token-chunk
NQ = 512               # attention q-chunk
NKT = S // 128         # 16 kv tiles per batch

_NC = None


def _split_multiwaits(nc, maxw=1):
    """Walrus codegen in this container rejects Drain instructions carrying
    more than ~2 semaphore waits ("Too many sync wait commands"). Move the
    excess waits onto preceding NoOps on the same engine."""
    ctr = 0
    for f in nc.m.functions:
        for bb in f.blocks:
            newlist = []
            changed = False
            for inst in bb.instructions:
                si = inst.sync_info
                if (si is not None and si.on_wait and len(si.on_wait) > maxw):
                    waits = list(si.on_wait)
                    for j in range(maxw, len(waits), maxw):
                        nop = mybir.InstNoOp(name=f"splitw-{ctr}", ins=[], outs=[])
                        ctr += 1
                        nop.engine = inst.engine
                        nop.sync_info = mybir.SyncInfo(
                            on_wait=list(waits[j:j + maxw]), on_update=[])
                        newlist.append(nop)
                    inst.sync_info = mybir.SyncInfo(
                        on_wait=waits[:maxw], on_update=list(si.on_update))
                    changed = True
                newlist.append(inst)
            if changed:
                bb.instructions = newlist
    return ctr


def _build(split=True):
    nc = bass.Bass()

    qT = nc.declare_dram_parameter("qT", [D, T], BF16, isOutput=False)
    kT = nc.declare_dram_parameter("kT", [D, T], BF16, isOutput=False)
    vT = nc.declare_dram_parameter("vT", [D, T], BF16, isOutput=False)
    wq = nc.declare_dram_parameter("wq", [D, HC], BF16, isOutput=False)
    wk = nc.declare_dram_parameter("wk", [D, HC], BF16, isOutput=False)
    wv = nc.declare_dram_parameter("wv", [D, HC], BF16, isOutput=False)
    bq = nc.declare_dram_parameter("bq", [HC, 1], F32, isOutput=False)
    bk = nc.declare_dram_parameter("bk", [HC, 1], F32, isOutput=False)
    wo = nc.declare_dram_parameter("wo", [HC, D], BF16, isOutput=False)
    idsw = nc.declare_dram_parameter("idsw", [128, 128], BF16, isOutput=False)
    out = nc.declare_dram_parameter("out", [T, D], BF16, isOutput=True)

    qT3 = qT.rearrange("(ko p) n -> p ko n", p=128)
    kT3 = kT.rearrange("(ko p) n -> p ko n", p=128)
    vT3 = vT.rearrange("(ko p) n -> p ko n", p=128)
    wq3 = wq.rearrange("(ko p) m -> p ko m", p=128)
    wk3 = wk.rearrange("(ko p) m -> p ko m", p=128)
    wv3 = wv.rearrange("(ko p) m -> p ko m", p=128)

    with tile.TileContext(nc) as tc:
        with (
            tc.tile_pool(name="consts", bufs=1) as consts,
            tc.tile_pool(name="big", bufs=1) as big,
            tc.tile_pool(name="qin", bufs=3) as qin_p,
            tc.tile_pool(name="kin", bufs=3) as kin_p,
            tc.tile_pool(name="vin", bufs=3) as vin_p,
            tc.tile_pool(name="st", bufs=4) as st_p,
            tc.tile_pool(name="e", bufs=6) as e_p,
            tc.tile_pool(name="rb", bufs=2) as rb_p,
            tc.tile_pool(name="os", bufs=3) as os_p,
            tc.tile_pool(name="pp", bufs=2, space="PSUM") as pp_p,
            tc.tile_pool(name="sc", bufs=3, space="PSUM") as sc_p,
            tc.tile_pool(name="av", bufs=3, space="PSUM") as av_p,
        ):
            # ---- persistent SBUF state ----
            wq_s = consts.tile([128, KO, 128], BF16, tag="wq")
            wk_s = consts.tile([128, KO, 128], BF16, tag="wk")
            wv_s = consts.tile([128, KO, 128], BF16, tag="wv")
            wo_s = consts.tile([HC, D], BF16, tag="wo")
            bq_s = consts.tile([HC, 1], F32, tag="bq")
            bk_s = consts.tile([HC, 1], F32, tag="bk")
            id_s = consts.tile([128, 128], BF16, tag="idsw")
            def _load_weights_front():
                nc.sync.dma_start(wk_s[:], wk3[:])
                nc.sync.dma_start(bk_s[:], bk[:])
            def _load_weights_rest():
                nc.sync.dma_start(wq_s[:], wq3[:])
                nc.sync.dma_start(bq_s[:], bq[:])
                nc.sync.dma_start(wv_s[:], wv3[:])
                nc.sync.dma_start(wo_s[:], wo[:])
                nc.sync.dma_start(id_s[:], idsw[:])

            # Per-head Q/K buffers zero-padded to 128 partitions: a K=64
            # matmul runs at half the K=128 streaming rate on this silicon,
            # so scores contract over 128 rows with rows 64-127 always zero.
            QTp = [big.tile([128, T], BF16, tag=f"QTp{h}", name=f"QTp{h}")
                   for h in range(2)]
            KTp = [big.tile([128, T], BF16, tag=f"KTp{h}", name=f"KTp{h}")
                   for h in range(2)]
            for h in range(2):
                nc.gpsimd.memset(QTp[h][64:128, :], 0.0)
                nc.gpsimd.memset(KTp[h][64:128, :], 0.0)
            attnT = big.tile([HC, T], BF16, tag="attnT")
            # [V_h0 | ones | V_h1] per (batch, k-tile): k-tokens on
            # partitions; head h's AV matmul takes the 128-wide block at
            # column offset 64*h, so the shared ones block lands on the
            # partitions opposite the head's AV rows and accumulates the
            # softmax denominator pre-broadcast across 64 partitions.
            V1 = [big.tile([128, NKT, 192], BF16, tag=f"V1_{b}",
                           name=f"V1_{b}") for b in range(B)]
            for b in range(B):
                nc.gpsimd.memset(V1[b][:, :, 64:128], 1.0)

            # ---- unit emitters -------------------------------------------
            ins_t = {"q": (qT3, qin_p, "qin"), "k": (kT3, kin_p, "kin"),
                     "v": (vT3, vin_p, "vin")}
            staged = {}

            def L(t, c):
                src, pool, tag = ins_t[t]
                buf = pool.tile([128, KO, CH], BF16, tag=tag,
                                name=f"{t}in{c}")
                nc.sync.dma_start(buf[:], src[:, :, bass.ts(c, CH)])
                staged[(t, c)] = buf

            def PQK(t, c):
                """Q or K projection of chunk c into QTp/KTp (+bias)."""
                buf = staged.pop((t, c))
                w_s, b_s, dst = ((wq_s, bq_s, QTp) if t == "q"
                                 else (wk_s, bk_s, KTp))
                ps = pp_p.tile([HC, CH], F32, tag="pp", name=f"ps_{t}{c}")
                for ko in range(KO):
                    nc.tensor.matmul(ps[:], w_s[:, ko, :], buf[:, ko, :],
                                     start=(ko == 0), stop=(ko == KO - 1))
                stg = st_p.tile([128, CH], BF16, tag="st", name=f"st_{t}{c}")
                nc.vector.tensor_scalar_add(stg[:], ps[:], b_s[:, 0:1])
                cs = bass.ts(c, CH)
                nc.vector.tensor_copy(dst[0][0:64, cs], stg[0:64, :])
                # partition shift 64-127 -> 0-63 via sbuf DMA
                nc.gpsimd.dma_start(dst[1][0:64, cs], stg[64:128, :])

            def PV(c):
                """V projection of chunk c into V1 (no bias: bv's
                contribution is folded into the host-side constant)."""
                buf = staged.pop(("v", c))
                b, ktb = (c * CH) // S, ((c * CH) % S) // 128
                for sub in range(4):
                    ps = pp_p.tile([128, 128], F32, tag="pp",
                                   name=f"ps_v{c}{sub}")
                    for ko in range(KO):
                        nc.tensor.matmul(ps[:], buf[:, ko, bass.ts(sub, 128)],
                                         wv_s[:, ko, :],
                                         start=(ko == 0), stop=(ko == KO - 1))
                    kt = ktb + sub
                    nc.vector.tensor_copy(V1[b][:, kt, 0:64], ps[:, 0:64])
                    nc.vector.tensor_copy(V1[b][:, kt, 128:192], ps[:, 64:128])

            def FC(b, qc, part):
                """Output-projection partial for token tiles of q-chunk."""
                for tl in (0, 1) if part == 0 else (2, 3):
                    tok0 = b * S + qc * NQ + tl * 128
                    osb = os_p.tile([128, D], BF16, tag="os",
                                    name=f"os{tok0}")
                    for half in range(2):
                        fp = pp_p.tile([128, 512], F32, tag="pp",
                                       name=f"fp{tok0}_{half}")
                        nc.tensor.matmul(fp[:], attnT[:, bass.ds(tok0, 128)],
                                         wo_s[:, bass.ts(half, 512)],
                                         start=True, stop=True)
                        nc.vector.tensor_copy(osb[:, bass.ts(half, 512)],
                                              fp[:])
                    nc.gpsimd.dma_start(out[bass.ds(tok0, 128), :], osb[:])

            def NORM(b, qc, av):
                """softmax normalization, fully on-chip: av[0] rows 64-127
                and av[1] rows 0-63 hold the denominators pre-broadcast;
                invert lane-parallel, half-swap onto the head's partitions
                with one identity matmul, multiply into attnT."""
                q0 = b * S + qc * NQ
                lnd = rb_p.tile([128, NQ], F32, tag="lnd",
                                name=f"lnd{b}{qc}")
                nc.scalar.activation(lnd[64:128, :], av[0][64:128, :], AF.Ln)
                nc.scalar.activation(lnd[0:64, :], av[1][0:64, :], AF.Ln)
                rb = rb_p.tile([128, NQ], BF16, tag="rb",
                               name=f"rb{b}{qc}")
                nc.scalar.activation(rb[:], lnd[:], AF.Exp, scale=-1.0)
                rbs = sc_p.tile([128, NQ], F32, tag="sc", name=f"rbs{b}{qc}")
                nc.tensor.matmul(rbs[:], id_s[:], rb[:], start=True, stop=True)
                rbc = rb_p.tile([128, NQ], BF16, tag="rbc",
                                name=f"rbc{b}{qc}")
                nc.vector.tensor_copy(rbc[:], rbs[:])
                nc.vector.tensor_mul(attnT[0:64, bass.ds(q0, NQ)],
                                     av[0][0:64, :], rbc[0:64, :])
                nc.vector.tensor_mul(attnT[64:128, bass.ds(q0, NQ)],
                                     av[1][64:128, :], rbc[64:128, :])

            def B_iter(b, qc, fillers):
                """Attention for one (batch, 512-token q-chunk); `fillers`
                is a list of up to 4 closures spliced at kt 2/6/10/14."""
                q0 = b * S + qc * NQ
                av = [av_p.tile([128, NQ], F32, tag="av",
                                name=f"av{b}{qc}{h}") for h in range(2)]
                e_prev = [None, None]
                for kt in range(NKT + 1):
                    if kt in (2, 6, 10, 14):
                        i = (2, 6, 10, 14).index(kt)
                        if i < len(fillers) and fillers[i] is not None:
                            fillers[i]()
                    e_cur = [None, None]
                    if kt < NKT:
                        for h in range(2):
                            sp = sc_p.tile([128, NQ], F32, tag="sc",
                                           name=f"sp{b}{qc}{kt}{h}")
                            nc.tensor.matmul(
                                sp[:], KTp[h][:, bass.ds(b * S + kt * 128, 128)],
                                QTp[h][:, bass.ds(q0, NQ)],
                                start=True, stop=True)
                            et = e_p.tile([128, NQ], BF16, tag="e",
                                          name=f"et{b}{qc}{kt}{h}")
                            nc.scalar.activation(et[:], sp[:], AF.Exp,
                                                 scale=SCALE)
                            e_cur[h] = et
                    if kt >= 1:
                        j = kt - 1
                        for h in range(2):
                            nc.tensor.matmul(
                                av[h][:], V1[b][:, j, bass.ds(64 * h, 128)],
                                e_prev[h][:],
                                start=(j == 0), stop=(j == NKT - 1))
                    e_prev = e_cur
                NORM(b, qc, av)

            # ---- pipelined emission --------------------------------------
            L("k", 0)
            _load_weights_front()
            L("k", 1)
            L("k", 2)
            _load_weights_rest()
            L("k", 3)
            L("q", 0)
            L("v", 0)
            L("v", 1)
            for c in (0, 1, 2, 3):
                PQK("k", c)
            PQK("q", 0)
            PV(0)
            L("v", 2)

            def u(*fns):
                def run():
                    for f in fns:
                        f()
                return run

            B_iter(0, 0, [
                u(lambda: PV(1), lambda: L("v", 3)),
                u(lambda: PV(2)),
                u(lambda: L("q", 1), lambda: PQK("q", 1), lambda: PV(3)),
                u(lambda: L("k", 4), lambda: L("k", 5)),
            ])
            B_iter(0, 1, [
                u(lambda: PQK("k", 4)),
                u(lambda: FC(0, 0, 0), lambda: L("k", 6)),
                u(lambda: FC(0, 0, 1)),
                u(lambda: L("q", 2), lambda: PQK("q", 2)),
            ])
            B_iter(0, 2, [
                u(lambda: PQK("k", 5), lambda: L("v", 4)),
                u(lambda: FC(0, 1, 0), lambda: L("k", 7)),
                u(lambda: FC(0, 1, 1), lambda: L("v", 5)),
                u(lambda: L("q", 3), lambda: PQK("q", 3)),
            ])
            B_iter(0, 3, [
                u(lambda: PQK("k", 6), lambda: PV(4)),
                u(lambda: PQK("k", 7), lambda: PV(5)),
                u(lambda: L("v", 6), lambda: FC(0, 2, 0), lambda: L("q", 4)),
                u(lambda: PQK("q", 4), lambda: FC(0, 2, 1)),
            ])
            B_iter(1, 0, [
                u(lambda: PV(6), lambda: L("v", 7)),
                u(lambda: PV(7), lambda: L("q", 5)),
                u(lambda: FC(0, 3, 0)),
                u(lambda: FC(0, 3, 1), lambda: PQK("q", 5)),
            ])
            B_iter(1, 1, [
                u(lambda: FC(1, 0, 0), lambda: L("q", 6)),
                u(lambda: FC(1, 0, 1)),
                u(lambda: PQK("q", 6)),
                None,
            ])
            B_iter(1, 2, [
                u(lambda: FC(1, 1, 0)),
                u(lambda: FC(1, 1, 1)),
                u(lambda: L("q", 7), lambda: PQK("q", 7)),
                None,
            ])
            B_iter(1, 3, [
                u(lambda: FC(1, 2, 0)),
                u(lambda: FC(1, 2, 1)),
                None,
                None,
            ])
            FC(1, 3, 0)
            FC(1, 3, 1)

    if split:
        _split_multiwaits(nc)
    return nc


def _get_nc():
    global _NC
    if _NC is None:
        _NC = _build()
    return _NC


def _prep_in_maps(q, k, v, Wq, bq, Wk, bk, Wv, bv, Wo, bo):
    bf = ml_dtypes.bfloat16
    qT = np.ascontiguousarray(q.reshape(T, D).T).astype(bf)
    kT = np.ascontiguousarray(k.reshape(T, D).T).astype(bf)
    vT = np.ascontiguousarray(v.reshape(T, D).T).astype(bf)
    idsw = np.roll(np.eye(128, dtype=np.float32), 64, axis=0).astype(bf)
    in_maps = []
    for c in range(NCORES):
        rows = slice(c * HC, (c + 1) * HC)
        in_maps.append({
            "qT": qT, "kT": kT, "vT": vT, "idsw": idsw,
            "wq": np.ascontiguousarray(Wq[rows, :].T).astype(bf),
            "wk": np.ascontiguousarray(Wk[rows, :].T).astype(bf),
            "wv": np.ascontiguousarray(Wv[rows, :].T).astype(bf),
            "bq": np.ascontiguousarray(bq[rows]).astype(np.float32).reshape(HC, 1),
            "bk": np.ascontiguousarray(bk[rows]).astype(np.float32).reshape(HC, 1),
            "wo": np.ascontiguousarray(Wo[:, rows].T).astype(bf),
        })
    return in_maps


def _run(inputs, trace=False):
    inputs = {k_: np.asarray(v_) for k_, v_ in inputs.items()}
    nc = _get_nc()
    in_maps = _prep_in_maps(**inputs)
    res = run_bass_kernel_spmd(nc, in_maps, core_ids=list(range(NCORES)),
                               trace=trace)
    acc = np.zeros((T, D), np.float64)
    for c in range(NCORES):
        acc += res.results[c]["out"].astype(np.float64)
    const = (inputs["bo"].astype(np.float64)
             + inputs["bv"].astype(np.float64) @ inputs["Wo"].astype(np.float64).T)
    acc += const[None, :]
    return acc.reshape(B, S, D).astype(np.float32), res


def kernel(**inputs) -> np.ndarray:
    return _run(inputs)[0]
